# revision 22
# baseline (speedup 1.0000x reference)
"""Multi-head self-attention Trainium2 kernel, 8-core SPMD.

Sharding: data-parallel over batch (2) x tensor-parallel over heads
(16 heads -> 4 per core).  Core c handles batch c//4, heads
[4*(c%4), 4*(c%4)+4).  Each core computes its 4 heads' attention and a
partial output projection; the host sums the 4 partials per batch and
adds the output bias.

Device kernel (per core), all weights/activations pre-transposed and
pre-sharded on the host:
  - Inputs land via a handful of large batched DMAs (HWDGE issue is
    ~0.6us per dma_start on the issuing engine), split between the Sync
    and Scalar HWDGE queues, ordered so the Q/K projection operands
    land first.
  - Tiny warmup matmuls + a dummy exp run during the DMA lead-in so
    the PE HAM clock-gate warms and the ACT exp table loads early.
  - Q,K projections produce feature-major [d, tokens] tiles (f16),
    V token-major [tokens, d] tiles (bf16) with an appended ones column
    (bias added via a precomputed broadcast tile, not a matmul).
  - Scores are computed transposed, S_T[k,q] = K.T@Q on the PE with two
    heads packed into disjoint row groups (d=64 contraction), issued in
    2-ktile bursts to reduce PE weight-switch exposure vs the AV MMs.
  - exp mostly via ScalarE (scale=1/8 folded in; no max subtraction --
    scores are N(0,1)-ish so exp is safe), written as bf16 P tiles.
    3 of 16 k-tiles per call use a DVE Schraudolph exp instead
    (u16 = round(a*s + b) bitcast to bf16, ~+-3.3% on P, which the
    softmax ratio mostly cancels) to offload the saturated ACT engine.
  - AV accumulates over k-tiles with lhsT = [V | ones], so PSUM row 64
    carries the softmax denominator for free; normalization uses a
    K=1 fp16 ones-matmul broadcast + fast DVE reciprocal.  AV lags the
    score/exp stream by one burst and each attention call's epilogue
    is deferred into the next call's first burst so the ACT exp stream
    never waits on the epilogue chain.
  - Projection/output-projection work is injected into attention
    burst slots ("extras") to fill PE slack under the ACT-bound stream.

Dtypes: fp16 on the projection/score path, bf16 on the exp/AV path,
fp32 PSUM accumulation everywhere.
"""

import numpy as np

import concourse.bacc as bacc
import concourse.bass as bass
import concourse.mybir as mybir
import concourse.tile as tile
from concourse.bass_utils import run_bass_kernel_spmd

F32 = mybir.dt.float32
I16 = mybir.dt.int16
BF16 = mybir.dt.bfloat16
FP16 = mybir.dt.float16
EXP = mybir.ActivationFunctionType.Exp
MULT = mybir.AluOpType.mult
ADD = mybir.AluOpType.add

DT = FP16     # projection / score path
DTF = BF16    # exp output P, V tiles
NP_DT = np.float16

N_CORES = 8
S = 2048          # tokens per batch
D = 1024          # d_model
NKT = 16          # 128-token k tiles
NQB = 4           # 512-token q blocks
NKD = 8           # 128-feature contraction tiles of d_model

# Schraudolph exp on DVE for these k-tiles (per attention call)
DVE_KTS = (3, 8, 13)
SCHRAU_A = (128.0 / np.log(2.0)) / 8.0          # folds the 1/sqrt(d) scale
SCHRAU_B = 127.0 * 128.0 - 5.7                  # bias + sawtooth centering

_CACHE = {}


def build(n_cores=N_CORES):
    nc = bacc.Bacc("TRN2", target_bir_lowering=False, num_devices=n_cores)
    xb0a = nc.declare_dram_parameter("xb0a", [128, 2048], DT, isOutput=False)
    xb0b = nc.declare_dram_parameter("xb0b", [128, 2048], DT, isOutput=False)
    xb1 = nc.declare_dram_parameter("xb1", [128, 4096], DT, isOutput=False)
    xb2 = nc.declare_dram_parameter("xb2", [128, 4096], DT, isOutput=False)
    xb3 = nc.declare_dram_parameter("xb3", [128, 4096], DT, isOutput=False)
    wqkA = nc.declare_dram_parameter("wqkA", [128, 2048], DT, isOutput=False)
    wqkB = nc.declare_dram_parameter("wqkB", [128, 2048], DT, isOutput=False)
    wvb = nc.declare_dram_parameter("wvb", [128, 2048], DT, isOutput=False)
    wob = nc.declare_dram_parameter("wob", [128, 2048], DT, isOutput=False)
    bqkT = nc.declare_dram_parameter("bqkT", [128, 4], F32, isOutput=False)
    bv = nc.declare_dram_parameter("bv", [1, 256], DT, isOutput=False)
    out = nc.declare_dram_parameter("out", [128, 16, 1024], DT, isOutput=True)

    with tile.TileContext(nc) as tc:
        with tc.tile_pool(name="const", bufs=1) as const, \
             tc.tile_pool(name="projp", bufs=2, space="PSUM") as projp, \
             tc.tile_pool(name="scp", bufs=2, space="PSUM") as scp, \
             tc.tile_pool(name="avp", bufs=2, space="PSUM") as avp, \
             tc.tile_pool(name="pp", bufs=9) as pp, \
             tc.tile_pool(name="dnp", bufs=2) as dnp, \
             tc.tile_pool(name="outsb", bufs=3) as outsb:

            # ---- warmup: HAM clock-gate + ACT exp-table preload during
            # the DMA lead-in.
            wtile = const.tile([64, 256], DT, name="warm_w")
            nc.vector.memset(wtile, 0.125)
            wdummy = const.tile([1, 2], DTF, name="warm_act")
            nc.scalar.activation(out=wdummy, in_=wtile[0:1, 0:2], func=EXP)
            wps = projp.tile([128, 512], F32, tag="proj", name="warm_ps")
            for i in range(6):
                nc.tensor.matmul(wps[0:64, 0:64], wtile[:, 0:64],
                                 wtile[:, 0:64], start=True, stop=True,
                                 skip_group_check=True)
            for i in range(4):
                nc.tensor.matmul(wps[0:64, 0:256], wtile[:, 0:64], wtile,
                                 start=True, stop=True, skip_group_check=True)

            # ---- input DMAs, batched
            xa_sb = const.tile([128, 2048], DT, name="xa", tag="xa")
            xb_sb = const.tile([128, 2048], DT, name="xb", tag="xb")
            x1_sb = const.tile([128, 4096], DT, name="x1", tag="x1")
            x2_sb = const.tile([128, 4096], DT, name="x2", tag="x2")
            x3_sb = const.tile([128, 4096], DT, name="x3", tag="x3")
            wqkA_sb = const.tile([128, 2048], DT, name="wqkA", tag="wqkA")
            wqkB_sb = const.tile([128, 2048], DT, name="wqkB", tag="wqkB")
            wv_sb = const.tile([128, 2048], DT, name="wv", tag="wv")
            wo_sb2 = const.tile([128, 2048], DT, name="wo", tag="wo")
            bqk_sb = const.tile([128, 4], F32, name="bqk_sb")
            bv_sb = const.tile([1, 256], DT, name="bv_sb")

            # Concurrently queued DMAs share SDMA bandwidth round-robin, so
            # gate later phases on earlier tiles via tiny copies: each phase
            # then gets ~full bandwidth and the early operands land first.
            nc.sync.dma_start(out=wqkA_sb, in_=wqkA.ap())
            nc.sync.dma_start(out=xa_sb, in_=xb0a.ap())
            nc.sync.dma_start(out=xb_sb, in_=xb0b.ap())
            nc.sync.dma_start(out=wv_sb, in_=wvb.ap())
            nc.gpsimd.tensor_copy(out=x1_sb[0:1, 0:1], in_=xb_sb[0:1, 0:1])
            nc.sync.dma_start(out=x1_sb, in_=xb1.ap())
            nc.gpsimd.tensor_copy(out=x2_sb[0:1, 0:1], in_=x1_sb[0:1, 0:1])
            nc.sync.dma_start(out=x2_sb, in_=xb2.ap())
            nc.gpsimd.tensor_copy(out=x3_sb[0:1, 0:1], in_=x2_sb[0:1, 0:1])
            nc.sync.dma_start(out=x3_sb, in_=xb3.ap())
            nc.gpsimd.tensor_copy(out=wqkB_sb[0:1, 0:1], in_=x2_sb[0:1, 0:1])
            nc.sync.dma_start(out=wqkB_sb, in_=wqkB.ap())
            nc.gpsimd.tensor_copy(out=wo_sb2[0:1, 0:1], in_=x2_sb[0:1, 0:1])
            nc.sync.dma_start(out=wo_sb2, in_=wob.ap())
            # scalar queue (issues after the exp-table preload): small items
            nc.scalar.dma_start(out=bqk_sb, in_=bqkT.ap())
            nc.scalar.dma_start(out=bv_sb, in_=bv.ap())

            def x_rhs(k, nb):
                if nb == 0:
                    t = xa_sb if k < 4 else xb_sb
                    return t[:, (k % 4) * 512:(k % 4 + 1) * 512]
                if nb == 1:
                    return x1_sb[:, k * 512:(k + 1) * 512]
                return (x2_sb if nb == 2 else x3_sb)[:, k * 512:(k + 1) * 512]

            def x_tok(k, t):
                r = x_rhs(k, t // 4)
                return r[:, (t % 4) * 128:(t % 4 + 1) * 128]

            WQK_VIEW = {2: (0, 0), 0: (0, 1), 3: (1, 0), 1: (1, 1)}

            def wqk_tile(m, k):
                t, j = WQK_VIEW[m]
                o = j * 1024 + k * 128
                return (wqkA_sb if t == 0 else wqkB_sb)[:, o:o + 128]

            ones_r = const.tile([1, 128], DT, name="ones_r")
            nc.vector.memset(ones_r, 1.0)

            # v bias broadcast tile: [128, 256] = ones.T @ bv
            bvb_sb = const.tile([128, 256], DTF, name="bvb")

            def bvb_init():
                bvb_ps = projp.tile([128, 512], F32, tag="proj", name="bvb_ps")
                nc.tensor.matmul(bvb_ps[:, 0:256], ones_r, bv_sb,
                                 start=True, stop=True)
                nc.vector.tensor_copy(out=bvb_sb, in_=bvb_ps[:, 0:256])

            v_sb = [const.tile([128, 4, 68], DTF, name=f"v{t}") for t in range(NKT)]
            for t in range(NKT):
                nc.vector.memset(v_sb[t][:, :, 64:65], 1.0)

            qk_sb = [const.tile([128, S], DT, name=f"qk{m}") for m in range(4)]
            av_sb = [const.tile([128, S], DT, name=f"av{p}") for p in range(2)]

            def qk_group(m, nb):
                ps = projp.tile([128, 512], F32, tag="proj", name="ps_qk")
                for k in range(NKD):
                    nc.tensor.matmul(
                        ps, wqk_tile(m, k), x_rhs(k, nb),
                        start=(k == 0), stop=(k == NKD - 1),
                    )
                nc.vector.tensor_scalar_add(
                    out=qk_sb[m][:, nb * 512:(nb + 1) * 512],
                    in0=ps,
                    scalar1=bqk_sb[:, m:m + 1],
                )

            def v_group(t):
                ps = projp.tile([128, 512], F32, tag="proj", name="ps_v")
                psv = ps[:, 0:256]
                for k in range(NKD):
                    nc.tensor.matmul(
                        psv, x_tok(k, t), wv_sb[:, k * 256:(k + 1) * 256],
                        start=(k == 0), stop=(k == NKD - 1),
                    )
                nc.vector.tensor_add(
                    out=v_sb[t][:, :, 0:64], in0=psv, in1=bvb_sb)

            def attn(pair, qb, extras=None, vjit=False, dve_kts=DVE_KTS):
                """Scores+exp+AV for one (head-pair, q-block).  Extras are
                keyed by k-tile (0..15), issued right after that k-tile's
                exp.  AV lags by one 2-ktile burst.  Returns the epilogue
                closure; the caller schedules it early in the next call."""
                avs = [
                    avp.tile([65, 512], F32, tag="av", name="avA"),
                    avp.tile([65, 512], F32, tag="av", name="avB"),
                ]
                qs = qk_sb[pair]
                extras = dict(extras or {})
                pts = []

                def ks_at(kt):
                    return qk_sb[2 + pair], kt * 128

                def av_mms(kt):
                    for h in (0, 1):
                        nc.tensor.matmul(
                            avs[h],
                            v_sb[kt][:, 2 * pair + h, 0:65],
                            pts[kt][:, h, :],
                            start=(kt == 0),
                            stop=(kt == NKT - 1),
                        )

                for kt in range(NKT):
                    sc = scp.tile([128, 2, 512], F32, tag="sc", name="sc")
                    kst, koff = ks_at(kt)
                    for h in (0, 1):
                        nc.tensor.matmul(
                            sc[:, h, :],
                            kst[h * 64:(h + 1) * 64, koff:koff + 128],
                            qs[h * 64:(h + 1) * 64, qb * 512:(qb + 1) * 512],
                            start=True,
                            stop=True,
                        )
                    pt = pp.tile([128, 2, 512], DTF, tag="p", name="pt")
                    if kt in dve_kts:
                        nc.vector.tensor_scalar(
                            out=pt[:, :, :].bitcast(I16),
                            in0=sc[:, :, :],
                            scalar1=float(SCHRAU_A),
                            scalar2=float(SCHRAU_B),
                            op0=MULT, op1=ADD,
                        )
                    else:
                        nc.scalar.activation(out=pt, in_=sc, func=EXP,
                                             scale=0.125)
                    pts.append(pt)
                    if vjit and kt % 2 == 0:
                        v_group(kt)
                        v_group(kt + 1)
                    fn = extras.pop(kt, None)
                    if fn is not None:
                        fn()
                    if kt % 4 == 3:
                        # flush AV in 8-MM runs: fewer PE weight-switch
                        # boundaries vs per-burst flushing; 8 MMs (~1.7us)
                        # still fit under the 2-deep exp pipeline buffer.
                        for fk in range(max(kt - 5, 0), kt - 1):
                            av_mms(fk)
                for fk in (NKT - 2, NKT - 1):
                    av_mms(fk)

                def epilogue():
                    for h in (0, 1):
                        den = dnp.tile([1, 512], DT, tag=f"den{h}", name="den")
                        nc.vector.tensor_copy(out=den, in_=avs[h][64:65, :])
                        rcp = projp.tile([64, 512], F32, tag="proj", name="rcp")
                        nc.tensor.matmul(rcp, ones_r[:, 0:64], den,
                                         start=True, stop=True)
                        rc = dnp.tile([64, 512], F32, tag=f"rc{h}", name="rc")
                        nc.vector.reciprocal_approx_fast(out=rc, in_=rcp)
                        nc.vector.tensor_mul(
                            out=av_sb[pair][h * 64:(h + 1) * 64,
                                            qb * 512:(qb + 1) * 512],
                            in0=avs[h][0:64, :],
                            in1=rc,
                        )
                return epilogue

            def outproj_tt(qb, tt, act_copy=False):
                tglob = qb * 4 + tt
                if tglob % 2 == 0:
                    ob = outsb.tile([128, 2, 1024], DT, tag="osb", name="ob_sb")
                    outproj_tt.cur = ob
                else:
                    ob = outproj_tt.cur
                tok = tglob * 128
                for obh in (0, 1):
                    ps = projp.tile([128, 512], F32, tag="proj", name="ps_o")
                    for p in (0, 1):
                        nc.tensor.matmul(
                            ps,
                            av_sb[p][:, tok:tok + 128],
                            wo_sb2[:, p * 1024 + obh * 512:p * 1024 + (obh + 1) * 512],
                            start=(p == 0),
                            stop=(p == 1),
                        )
                    dst = ob[:, tglob % 2, obh * 512:(obh + 1) * 512]
                    if act_copy and obh == 1:
                        nc.scalar.copy(out=dst, in_=ps)
                    else:
                        nc.vector.tensor_copy(out=dst, in_=ps)
                if tglob % 2 == 1:
                    nc.sync.dma_start(
                        out=out.ap()[:, tglob - 1:tglob + 1, :], in_=ob)
            outproj_tt.cur = None

            qk_group(2, 0)               # K pair0 block 0
            qk_group(0, 0)               # Q pair0, qb0
            bvb_init()
            ep = attn(0, 0, extras={
                2: lambda: qk_group(2, 1), 6: lambda: qk_group(2, 2),
                10: lambda: qk_group(2, 3), 14: lambda: qk_group(0, 1),
            }, vjit=True)
            ep = attn(0, 1, extras={1: ep, 8: lambda: qk_group(0, 2)})
            ep = attn(0, 2, extras={1: ep, 8: lambda: qk_group(0, 3)})
            ep = attn(0, 3, extras={
                1: ep, 4: lambda: qk_group(3, 0), 9: lambda: qk_group(1, 0),
                13: lambda: qk_group(3, 1),
            })
            ep = attn(1, 0, extras={
                1: ep, 2: lambda: qk_group(3, 2), 6: lambda: qk_group(3, 3),
                12: lambda: qk_group(1, 1),
            })
            ep = attn(1, 1, extras={
                1: ep, 2: lambda: qk_group(1, 2),
                6: lambda: outproj_tt(0, 0), 9: lambda: outproj_tt(0, 1),
                12: lambda: outproj_tt(0, 2), 14: lambda: outproj_tt(0, 3),
            })
            ep = attn(1, 2, extras={
                1: ep, 2: lambda: qk_group(1, 3),
                6: lambda: outproj_tt(1, 0), 9: lambda: outproj_tt(1, 1),
                12: lambda: outproj_tt(1, 2), 14: lambda: outproj_tt(1, 3),
            })
            ep = attn(1, 3, extras={
                1: ep, 4: lambda: outproj_tt(2, 0), 6: lambda: outproj_tt(2, 1),
                10: lambda: outproj_tt(2, 2), 13: lambda: outproj_tt(2, 3),
            })
            ep()
            outproj_tt(3, 0, act_copy=True)
            outproj_tt(3, 1, act_copy=True)
            outproj_tt(3, 2, act_copy=True)
            outproj_tt(3, 3, act_copy=True)

    nc.compile()
    return nc


def make_in_maps(x, w_qkv, b_qkv, w_out):
    """Shard FULL inputs into per-core input dicts (host-side, free)."""
    x = np.ascontiguousarray(np.asarray(x, dtype=np.float32))
    w_qkv = np.asarray(w_qkv, dtype=np.float32)
    b_qkv = np.asarray(b_qkv, dtype=np.float32)
    w_out = np.asarray(w_out, dtype=np.float32)
    in_maps = []
    for c in range(N_CORES):
        b = c // 4
        g = c % 4
        r = 256 * g
        wq = w_qkv[r:r + 256]             # [256, 1024]
        wk = w_qkv[1024 + r:1024 + r + 256]
        wv = w_qkv[2048 + r:2048 + r + 256]
        bq = b_qkv[r:r + 256]
        bk = b_qkv[1024 + r:1024 + r + 256]
        bvv = b_qkv[2048 + r:2048 + r + 256]
        x_r = np.ascontiguousarray(
            x[b].T.reshape(8, 128, 2048).transpose(1, 0, 2)).astype(NP_DT)
        wqkT = np.concatenate([wq, wk], 0).T.astype(NP_DT)  # [1024, 512]
        wqk_r = wqkT.reshape(8, 128, 4, 128).transpose(1, 2, 0, 3)
        in_maps.append({
            "xb0a": np.ascontiguousarray(x_r[:, 0:4, 0:512].reshape(128, -1)),
            "xb0b": np.ascontiguousarray(x_r[:, 4:8, 0:512].reshape(128, -1)),
            "xb1": np.ascontiguousarray(x_r[:, :, 512:1024].reshape(128, -1)),
            "xb2": np.ascontiguousarray(x_r[:, :, 1024:1536].reshape(128, -1)),
            "xb3": np.ascontiguousarray(x_r[:, :, 1536:2048].reshape(128, -1)),
            "wqkA": np.ascontiguousarray(wqk_r[:, [2, 0]].reshape(128, -1)),
            "wqkB": np.ascontiguousarray(wqk_r[:, [3, 1]].reshape(128, -1)),
            "wvb": np.ascontiguousarray(
                wv.T.astype(NP_DT).reshape(8, 128, 256).transpose(1, 0, 2)
                .reshape(128, -1)),
            "wob": np.ascontiguousarray(
                w_out[:, r:r + 256].T.astype(NP_DT).reshape(2, 128, 1024)
                .transpose(1, 0, 2).reshape(128, -1)),
            "bqkT": np.ascontiguousarray(
                np.stack([bq[:128], bq[128:], bk[:128], bk[128:]], axis=1)
            ),
            "bv": np.ascontiguousarray(bvv[None, :]).astype(NP_DT),
        })
    return in_maps


def combine(results, b_out):
    """Sum per-core partials within each batch and add output bias."""
    b_out = np.asarray(b_out, dtype=np.float64)
    outs = []
    for b in range(2):
        acc = np.zeros((S, D), dtype=np.float64)
        for g in range(4):
            o = results[4 * b + g]["out"].astype(np.float64)  # [128, 16, 1024]
            acc += o.transpose(1, 0, 2).reshape(S, D)
        outs.append(acc + b_out)
    return np.stack(outs).astype(np.float32)


def kernel(x, w_qkv, b_qkv, w_out, b_out):
    if "nc" not in _CACHE:
        _CACHE["nc"] = build()
    nc = _CACHE["nc"]
    in_maps = make_in_maps(x, w_qkv, b_qkv, w_out)
    res = run_bass_kernel_spmd(nc, in_maps, list(range(N_CORES)))
    return combine(res.results, b_out)


# revision 23
# speedup vs baseline: 1.0047x; 1.0047x over previous
"""Multi-head self-attention Trainium2 kernel, 8-core SPMD.

Sharding: data-parallel over batch (2) x tensor-parallel over heads
(16 heads -> 4 per core).  Core c handles batch c//4, heads
[4*(c%4), 4*(c%4)+4).  Each core computes its 4 heads' attention and a
partial output projection; the host sums the 4 partials per batch and
adds the output bias.

Device kernel (per core), all weights/activations pre-transposed and
pre-sharded on the host:
  - Inputs land via a handful of large batched DMAs (HWDGE issue is
    ~0.6us per dma_start on the issuing engine), split between the Sync
    and Scalar HWDGE queues, ordered so the Q/K projection operands
    land first.
  - Tiny warmup matmuls + a dummy exp run during the DMA lead-in so
    the PE HAM clock-gate warms and the ACT exp table loads early.
  - Q,K projections produce feature-major [d, tokens] tiles (f16),
    V token-major [tokens, d] tiles (bf16) with an appended ones column
    (bias added via a precomputed broadcast tile, not a matmul).
  - Scores are computed transposed, S_T[k,q] = K.T@Q on the PE with two
    heads packed into disjoint row groups (d=64 contraction), issued in
    2-ktile bursts to reduce PE weight-switch exposure vs the AV MMs.
  - exp mostly via ScalarE (scale=1/8 folded in; no max subtraction --
    scores are N(0,1)-ish so exp is safe), written as bf16 P tiles.
    3 of 16 k-tiles per call use a DVE Schraudolph exp instead
    (u16 = round(a*s + b) bitcast to bf16, ~+-3.3% on P, which the
    softmax ratio mostly cancels) to offload the saturated ACT engine.
  - AV accumulates over k-tiles with lhsT = [V | ones], so PSUM row 64
    carries the softmax denominator for free; normalization uses a
    K=1 fp16 ones-matmul broadcast + fast DVE reciprocal.  AV flushes
    in 8-MM runs at kt 3/7/11/15 (lagging the score/exp stream by >=2
    k-tiles) and each attention call's epilogue is deferred into the
    next call's slot 1 so the ACT exp stream never waits on it.
  - Projection/output-projection work is injected into attention
    burst slots ("extras") to fill PE slack under the ACT-bound stream.

Dtypes: fp16 on the projection/score path, bf16 on the exp/AV path,
fp32 PSUM accumulation everywhere.
"""

import numpy as np

import concourse.bacc as bacc
import concourse.bass as bass
import concourse.mybir as mybir
import concourse.tile as tile
from concourse.bass_utils import run_bass_kernel_spmd

F32 = mybir.dt.float32
I16 = mybir.dt.int16
BF16 = mybir.dt.bfloat16
FP16 = mybir.dt.float16
EXP = mybir.ActivationFunctionType.Exp
MULT = mybir.AluOpType.mult
ADD = mybir.AluOpType.add

DT = FP16     # projection / score path
DTF = BF16    # exp output P, V tiles
NP_DT = np.float16

N_CORES = 8
S = 2048          # tokens per batch
D = 1024          # d_model
NKT = 16          # 128-token k tiles
NQB = 4           # 512-token q blocks
NKD = 8           # 128-feature contraction tiles of d_model

# Schraudolph exp on DVE for these k-tiles (per attention call)
DVE_KTS = (3, 8, 13)
SCHRAU_A = (128.0 / np.log(2.0)) / 8.0          # folds the 1/sqrt(d) scale
SCHRAU_B = 127.0 * 128.0 - 5.7                  # bias + sawtooth centering

_CACHE = {}


def build(n_cores=N_CORES):
    nc = bacc.Bacc("TRN2", target_bir_lowering=False, num_devices=n_cores)
    xb0a = nc.declare_dram_parameter("xb0a", [128, 2048], DT, isOutput=False)
    xb0b = nc.declare_dram_parameter("xb0b", [128, 2048], DT, isOutput=False)
    xb1 = nc.declare_dram_parameter("xb1", [128, 4096], DT, isOutput=False)
    xb2 = nc.declare_dram_parameter("xb2", [128, 4096], DT, isOutput=False)
    xb3 = nc.declare_dram_parameter("xb3", [128, 4096], DT, isOutput=False)
    wqkA = nc.declare_dram_parameter("wqkA", [128, 2048], DT, isOutput=False)
    wqkB = nc.declare_dram_parameter("wqkB", [128, 2048], DT, isOutput=False)
    wvb = nc.declare_dram_parameter("wvb", [128, 2048], DT, isOutput=False)
    wob = nc.declare_dram_parameter("wob", [128, 2048], DT, isOutput=False)
    bqkT = nc.declare_dram_parameter("bqkT", [128, 4], F32, isOutput=False)
    bv = nc.declare_dram_parameter("bv", [1, 256], DT, isOutput=False)
    out = nc.declare_dram_parameter("out", [128, 16, 1024], DT, isOutput=True)

    with tile.TileContext(nc) as tc:
        with tc.tile_pool(name="const", bufs=1) as const, \
             tc.tile_pool(name="projp", bufs=2, space="PSUM") as projp, \
             tc.tile_pool(name="scp", bufs=2, space="PSUM") as scp, \
             tc.tile_pool(name="avp", bufs=2, space="PSUM") as avp, \
             tc.tile_pool(name="pp", bufs=9) as pp, \
             tc.tile_pool(name="dnp", bufs=2) as dnp, \
             tc.tile_pool(name="outsb", bufs=3) as outsb:

            # ---- warmup: HAM clock-gate + ACT exp-table preload during
            # the DMA lead-in.
            wtile = const.tile([64, 256], DT, name="warm_w")
            nc.vector.memset(wtile, 0.125)
            wdummy = const.tile([1, 2], DTF, name="warm_act")
            nc.scalar.activation(out=wdummy, in_=wtile[0:1, 0:2], func=EXP)
            wps = projp.tile([128, 512], F32, tag="proj", name="warm_ps")
            for i in range(6):
                nc.tensor.matmul(wps[0:64, 0:64], wtile[:, 0:64],
                                 wtile[:, 0:64], start=True, stop=True,
                                 skip_group_check=True)
            for i in range(4):
                nc.tensor.matmul(wps[0:64, 0:256], wtile[:, 0:64], wtile,
                                 start=True, stop=True, skip_group_check=True)

            # ---- input DMAs, batched
            xa_sb = const.tile([128, 2048], DT, name="xa", tag="xa")
            xb_sb = const.tile([128, 2048], DT, name="xb", tag="xb")
            x1_sb = const.tile([128, 4096], DT, name="x1", tag="x1")
            x2_sb = const.tile([128, 4096], DT, name="x2", tag="x2")
            x3_sb = const.tile([128, 4096], DT, name="x3", tag="x3")
            wqkA_sb = const.tile([128, 2048], DT, name="wqkA", tag="wqkA")
            wqkB_sb = const.tile([128, 2048], DT, name="wqkB", tag="wqkB")
            wv_sb = const.tile([128, 2048], DT, name="wv", tag="wv")
            wo_sb2 = const.tile([128, 2048], DT, name="wo", tag="wo")
            bqk_sb = const.tile([128, 4], F32, name="bqk_sb")
            bv_sb = const.tile([1, 256], DT, name="bv_sb")

            # Concurrently queued DMAs share SDMA bandwidth round-robin, so
            # gate later phases on earlier tiles via tiny copies: each phase
            # then gets ~full bandwidth and the early operands land first.
            nc.sync.dma_start(out=wqkA_sb, in_=wqkA.ap())
            nc.sync.dma_start(out=xa_sb, in_=xb0a.ap())
            nc.sync.dma_start(out=xb_sb, in_=xb0b.ap())
            nc.sync.dma_start(out=wv_sb, in_=wvb.ap())
            nc.gpsimd.tensor_copy(out=x1_sb[0:1, 0:1], in_=xb_sb[0:1, 0:1])
            nc.sync.dma_start(out=x1_sb, in_=xb1.ap())
            nc.gpsimd.tensor_copy(out=x2_sb[0:1, 0:1], in_=x1_sb[0:1, 0:1])
            nc.sync.dma_start(out=x2_sb, in_=xb2.ap())
            nc.gpsimd.tensor_copy(out=x3_sb[0:1, 0:1], in_=x2_sb[0:1, 0:1])
            nc.sync.dma_start(out=x3_sb, in_=xb3.ap())
            nc.gpsimd.tensor_copy(out=wqkB_sb[0:1, 0:1], in_=x2_sb[0:1, 0:1])
            nc.sync.dma_start(out=wqkB_sb, in_=wqkB.ap())
            nc.gpsimd.tensor_copy(out=wo_sb2[0:1, 0:1], in_=x2_sb[0:1, 0:1])
            nc.sync.dma_start(out=wo_sb2, in_=wob.ap())
            # scalar queue (issues after the exp-table preload): small items
            nc.scalar.dma_start(out=bqk_sb, in_=bqkT.ap())
            nc.scalar.dma_start(out=bv_sb, in_=bv.ap())

            def x_rhs(k, nb):
                if nb == 0:
                    t = xa_sb if k < 4 else xb_sb
                    return t[:, (k % 4) * 512:(k % 4 + 1) * 512]
                if nb == 1:
                    return x1_sb[:, k * 512:(k + 1) * 512]
                return (x2_sb if nb == 2 else x3_sb)[:, k * 512:(k + 1) * 512]

            def x_tok(k, t):
                r = x_rhs(k, t // 4)
                return r[:, (t % 4) * 128:(t % 4 + 1) * 128]

            WQK_VIEW = {2: (0, 0), 0: (0, 1), 3: (1, 0), 1: (1, 1)}

            def wqk_tile(m, k):
                t, j = WQK_VIEW[m]
                o = j * 1024 + k * 128
                return (wqkA_sb if t == 0 else wqkB_sb)[:, o:o + 128]

            ones_r = const.tile([1, 128], DT, name="ones_r")
            nc.vector.memset(ones_r, 1.0)

            # v bias broadcast tile: [128, 256] = ones.T @ bv
            bvb_sb = const.tile([128, 256], DTF, name="bvb")

            def bvb_init():
                bvb_ps = projp.tile([128, 512], F32, tag="proj", name="bvb_ps")
                nc.tensor.matmul(bvb_ps[:, 0:256], ones_r, bv_sb,
                                 start=True, stop=True)
                nc.vector.tensor_copy(out=bvb_sb, in_=bvb_ps[:, 0:256])

            v_sb = [const.tile([128, 4, 68], DTF, name=f"v{t}") for t in range(NKT)]
            for t in range(NKT):
                nc.vector.memset(v_sb[t][:, :, 64:65], 1.0)

            qk_sb = [const.tile([128, S], DT, name=f"qk{m}") for m in range(4)]
            av_sb = [const.tile([128, S], DT, name=f"av{p}") for p in range(2)]

            def qk_group(m, nb):
                ps = projp.tile([128, 512], F32, tag="proj", name="ps_qk")
                for k in range(NKD):
                    nc.tensor.matmul(
                        ps, wqk_tile(m, k), x_rhs(k, nb),
                        start=(k == 0), stop=(k == NKD - 1),
                    )
                nc.vector.tensor_scalar_add(
                    out=qk_sb[m][:, nb * 512:(nb + 1) * 512],
                    in0=ps,
                    scalar1=bqk_sb[:, m:m + 1],
                )

            def v_group(t):
                ps = projp.tile([128, 512], F32, tag="proj", name="ps_v")
                psv = ps[:, 0:256]
                for k in range(NKD):
                    nc.tensor.matmul(
                        psv, x_tok(k, t), wv_sb[:, k * 256:(k + 1) * 256],
                        start=(k == 0), stop=(k == NKD - 1),
                    )
                nc.vector.tensor_add(
                    out=v_sb[t][:, :, 0:64], in0=psv, in1=bvb_sb)

            def attn(pair, qb, extras=None, vjit=False, dve_kts=DVE_KTS):
                """Scores+exp+AV for one (head-pair, q-block).  Extras are
                keyed by k-tile (0..15), issued right after that k-tile's
                exp.  AV lags by one 2-ktile burst.  Returns the epilogue
                closure; the caller schedules it early in the next call."""
                avs = [
                    avp.tile([65, 512], F32, tag="av", name="avA"),
                    avp.tile([65, 512], F32, tag="av", name="avB"),
                ]
                qs = qk_sb[pair]
                extras = dict(extras or {})
                pts = []

                def ks_at(kt):
                    return qk_sb[2 + pair], kt * 128

                def av_mms(kt):
                    for h in (0, 1):
                        nc.tensor.matmul(
                            avs[h],
                            v_sb[kt][:, 2 * pair + h, 0:65],
                            pts[kt][:, h, :],
                            start=(kt == 0),
                            stop=(kt == NKT - 1),
                        )

                for kt in range(NKT):
                    sc = scp.tile([128, 2, 512], F32, tag="sc", name="sc")
                    kst, koff = ks_at(kt)
                    for h in (0, 1):
                        nc.tensor.matmul(
                            sc[:, h, :],
                            kst[h * 64:(h + 1) * 64, koff:koff + 128],
                            qs[h * 64:(h + 1) * 64, qb * 512:(qb + 1) * 512],
                            start=True,
                            stop=True,
                        )
                    pt = pp.tile([128, 2, 512], DTF, tag="p", name="pt")
                    if kt in dve_kts:
                        nc.vector.tensor_scalar(
                            out=pt[:, :, :].bitcast(I16),
                            in0=sc[:, :, :],
                            scalar1=float(SCHRAU_A),
                            scalar2=float(SCHRAU_B),
                            op0=MULT, op1=ADD,
                        )
                    else:
                        nc.scalar.activation(out=pt, in_=sc, func=EXP,
                                             scale=0.125)
                    pts.append(pt)
                    if vjit and kt % 2 == 0:
                        v_group(kt)
                        v_group(kt + 1)
                    fn = extras.pop(kt, None)
                    if fn is not None:
                        fn()
                    if kt % 4 == 3:
                        # flush AV in 8-MM runs: fewer PE weight-switch
                        # boundaries vs per-burst flushing; 8 MMs (~1.7us)
                        # still fit under the 2-deep exp pipeline buffer.
                        for fk in range(max(kt - 5, 0), kt - 1):
                            av_mms(fk)
                for fk in (NKT - 2, NKT - 1):
                    av_mms(fk)

                def epilogue():
                    for h in (0, 1):
                        den = dnp.tile([1, 512], DT, tag=f"den{h}", name="den")
                        nc.vector.tensor_copy(out=den, in_=avs[h][64:65, :])
                        rcp = projp.tile([64, 512], F32, tag="proj", name="rcp")
                        nc.tensor.matmul(rcp, ones_r[:, 0:64], den,
                                         start=True, stop=True)
                        rc = dnp.tile([64, 512], F32, tag=f"rc{h}", name="rc")
                        nc.vector.reciprocal_approx_fast(out=rc, in_=rcp)
                        nc.vector.tensor_mul(
                            out=av_sb[pair][h * 64:(h + 1) * 64,
                                            qb * 512:(qb + 1) * 512],
                            in0=avs[h][0:64, :],
                            in1=rc,
                        )
                return epilogue

            def outproj_tt(qb, tt, act_copy=False):
                tglob = qb * 4 + tt
                if tglob % 2 == 0:
                    ob = outsb.tile([128, 2, 1024], DT, tag="osb", name="ob_sb")
                    outproj_tt.cur = ob
                else:
                    ob = outproj_tt.cur
                tok = tglob * 128
                for obh in (0, 1):
                    ps = projp.tile([128, 512], F32, tag="proj", name="ps_o")
                    for p in (0, 1):
                        nc.tensor.matmul(
                            ps,
                            av_sb[p][:, tok:tok + 128],
                            wo_sb2[:, p * 1024 + obh * 512:p * 1024 + (obh + 1) * 512],
                            start=(p == 0),
                            stop=(p == 1),
                        )
                    dst = ob[:, tglob % 2, obh * 512:(obh + 1) * 512]
                    if act_copy and obh == 1:
                        nc.scalar.copy(out=dst, in_=ps)
                    else:
                        nc.vector.tensor_copy(out=dst, in_=ps)
                if tglob % 2 == 1:
                    nc.sync.dma_start(
                        out=out.ap()[:, tglob - 1:tglob + 1, :], in_=ob)
            outproj_tt.cur = None

            qk_group(2, 0)               # K pair0 block 0
            qk_group(0, 0)               # Q pair0, qb0
            bvb_init()
            ep = attn(0, 0, extras={
                2: lambda: qk_group(2, 1), 6: lambda: qk_group(2, 2),
                10: lambda: qk_group(2, 3), 14: lambda: qk_group(0, 1),
            }, vjit=True)
            ep = attn(0, 1, extras={1: ep, 8: lambda: qk_group(0, 2)})
            ep = attn(0, 2, extras={1: ep, 8: lambda: qk_group(0, 3)})
            ep = attn(0, 3, extras={
                1: ep, 4: lambda: qk_group(3, 0), 9: lambda: qk_group(1, 0),
                13: lambda: qk_group(3, 1),
            })
            ep = attn(1, 0, extras={
                1: ep, 2: lambda: qk_group(3, 2), 6: lambda: qk_group(3, 3),
                12: lambda: qk_group(1, 1),
            })
            ep = attn(1, 1, extras={
                1: ep, 2: lambda: qk_group(1, 2),
                6: lambda: outproj_tt(0, 0), 9: lambda: outproj_tt(0, 1),
                12: lambda: outproj_tt(0, 2), 14: lambda: outproj_tt(0, 3),
            })
            ep = attn(1, 2, extras={
                1: ep, 2: lambda: qk_group(1, 3),
                6: lambda: outproj_tt(1, 0), 9: lambda: outproj_tt(1, 1),
                12: lambda: outproj_tt(1, 2), 14: lambda: outproj_tt(1, 3),
            })
            ep = attn(1, 3, extras={
                1: ep, 4: lambda: outproj_tt(2, 0), 6: lambda: outproj_tt(2, 1),
                10: lambda: outproj_tt(2, 2), 13: lambda: outproj_tt(2, 3),
            })
            ep()
            outproj_tt(3, 0, act_copy=True)
            outproj_tt(3, 1, act_copy=True)
            outproj_tt(3, 2, act_copy=True)
            outproj_tt(3, 3, act_copy=True)

    nc.compile()
    return nc


def make_in_maps(x, w_qkv, b_qkv, w_out):
    """Shard FULL inputs into per-core input dicts (host-side, free)."""
    x = np.ascontiguousarray(np.asarray(x, dtype=np.float32))
    w_qkv = np.asarray(w_qkv, dtype=np.float32)
    b_qkv = np.asarray(b_qkv, dtype=np.float32)
    w_out = np.asarray(w_out, dtype=np.float32)
    in_maps = []
    for c in range(N_CORES):
        b = c // 4
        g = c % 4
        r = 256 * g
        wq = w_qkv[r:r + 256]             # [256, 1024]
        wk = w_qkv[1024 + r:1024 + r + 256]
        wv = w_qkv[2048 + r:2048 + r + 256]
        bq = b_qkv[r:r + 256]
        bk = b_qkv[1024 + r:1024 + r + 256]
        bvv = b_qkv[2048 + r:2048 + r + 256]
        x_r = np.ascontiguousarray(
            x[b].T.reshape(8, 128, 2048).transpose(1, 0, 2)).astype(NP_DT)
        wqkT = np.concatenate([wq, wk], 0).T.astype(NP_DT)  # [1024, 512]
        wqk_r = wqkT.reshape(8, 128, 4, 128).transpose(1, 2, 0, 3)
        in_maps.append({
            "xb0a": np.ascontiguousarray(x_r[:, 0:4, 0:512].reshape(128, -1)),
            "xb0b": np.ascontiguousarray(x_r[:, 4:8, 0:512].reshape(128, -1)),
            "xb1": np.ascontiguousarray(x_r[:, :, 512:1024].reshape(128, -1)),
            "xb2": np.ascontiguousarray(x_r[:, :, 1024:1536].reshape(128, -1)),
            "xb3": np.ascontiguousarray(x_r[:, :, 1536:2048].reshape(128, -1)),
            "wqkA": np.ascontiguousarray(wqk_r[:, [2, 0]].reshape(128, -1)),
            "wqkB": np.ascontiguousarray(wqk_r[:, [3, 1]].reshape(128, -1)),
            "wvb": np.ascontiguousarray(
                wv.T.astype(NP_DT).reshape(8, 128, 256).transpose(1, 0, 2)
                .reshape(128, -1)),
            "wob": np.ascontiguousarray(
                w_out[:, r:r + 256].T.astype(NP_DT).reshape(2, 128, 1024)
                .transpose(1, 0, 2).reshape(128, -1)),
            "bqkT": np.ascontiguousarray(
                np.stack([bq[:128], bq[128:], bk[:128], bk[128:]], axis=1)
            ),
            "bv": np.ascontiguousarray(bvv[None, :]).astype(NP_DT),
        })
    return in_maps


def combine(results, b_out):
    """Sum per-core partials within each batch and add output bias."""
    b_out = np.asarray(b_out, dtype=np.float64)
    outs = []
    for b in range(2):
        acc = np.zeros((S, D), dtype=np.float64)
        for g in range(4):
            o = results[4 * b + g]["out"].astype(np.float64)  # [128, 16, 1024]
            acc += o.transpose(1, 0, 2).reshape(S, D)
        outs.append(acc + b_out)
    return np.stack(outs).astype(np.float32)


def kernel(x, w_qkv, b_qkv, w_out, b_out):
    if "nc" not in _CACHE:
        _CACHE["nc"] = build()
    nc = _CACHE["nc"]
    in_maps = make_in_maps(x, w_qkv, b_qkv, w_out)
    res = run_bass_kernel_spmd(nc, in_maps, list(range(N_CORES)))
    return combine(res.results, b_out)
